# revision 12
# baseline (speedup 1.0000x reference)
"""Trainium2 Bass kernel for nn_MultiHeadAttention_82446192214635 (v3).

Full inputs in, full output out. Sharding: 8 cores = 4 batches x 2 head-groups
(8 heads each). Each core computes its batch's attention for its 8 heads plus
the partial output projection; host sums the two head-group partials per batch
and adds bo (plus the folded bv@Wo term).

v3 changes vs the 682us v2 baseline:
  - Cross-pass pipelining: all tile pools are opened once (outside the
    repeat loop); kT2 is double-buffered, and pass n+1's k projection is
    emitted as PE fillers inside pass n's last s-chunk, where the PE has
    gaps while ACT/DVE drain the attention tail. The serial per-pass head
    (k-proj with ACT idle) disappears in steady state.
  - v projection is emitted as paced fillers inside the first attention
    pair of each pass (PV lags QK by 2 tiles; the pacing keeps the v tile
    for PV(tt) emitted before it is consumed), removing the other half of
    the serial head.
  - bk is dropped: scores' q~.bk term is constant over keys and cancels
    in softmax (exact). bv is dropped on device: attention weights sum to
    1, so bv contributes bv@Wo to the output, folded into bo on the host
    (exact). k/v projections drain PSUM with a plain tensor_copy.
  - PV drain fused: the unnormalized ctx rows are multiplied by the
    broadcast reciprocal directly out of PSUM (one tensor_mul per head
    instead of copy+mul).

Kept from v2: host-side bf16 casts and transposes, loop-invariant consts
loaded once, exp(bias^T) precomputed once, attention inner loop software-
pipelined by two tiles, step-0 repeat AP for the pair bias multiply, DMA
queue split (sync/scalar/gpsimd), reciprocal_approx_fast + gpsimd
partition_broadcast normalization (custom DVE recip reads garbage from
PSUM on HW; gpsimd cannot read PSUM).
"""

import numpy as np

B, S, E = 4, 2048, 1024
H, DH = 16, 64
HL = 8          # heads per core
DL = HL * DH    # 512
N_CORES = 8
ST = S // 128   # 16 t-tiles
ES = E // 128   # 8 e-strips
SC = S // 512   # 4 s-chunks
NP = HL // 2    # 4 head pairs

_NC_CACHE = {}


def build_nc(repeat=1):
    from collections import deque
    import concourse.bass as bass
    import concourse.tile as tile
    from concourse import bacc, mybir

    f32 = mybir.dt.float32
    bf16 = mybir.dt.bfloat16
    Exp = mybir.ActivationFunctionType.Exp

    nc = bacc.Bacc("TRN2", target_bir_lowering=False, debug=False,
                   num_devices=N_CORES)

    qT_d = nc.dram_tensor("qt", [E, S], bf16, kind="ExternalInput")
    kT_d = nc.dram_tensor("kt", [E, S], bf16, kind="ExternalInput")
    vT_d = nc.dram_tensor("vt", [E, S], bf16, kind="ExternalInput")
    biasT_d = nc.dram_tensor("biast", [S, S], bf16, kind="ExternalInput")
    wq_d = nc.dram_tensor("wq", [E, DL], bf16, kind="ExternalInput")
    wk_d = nc.dram_tensor("wk", [E, DL], bf16, kind="ExternalInput")
    wv_d = nc.dram_tensor("wv", [E, DL], bf16, kind="ExternalInput")
    wo_d = nc.dram_tensor("wo", [DL, E], bf16, kind="ExternalInput")
    bq_d = nc.dram_tensor("bq", [DL], f32, kind="ExternalInput")
    out_d = nc.dram_tensor("out", [S, E], f32, kind="ExternalOutput")

    with tile.TileContext(nc) as tc:
        with (
            tc.tile_pool(name="consts", bufs=1) as consts,
            tc.tile_pool(name="persist", bufs=1) as persist,
            tc.tile_pool(name="kT2p", bufs=2) as kT2p,
            tc.tile_pool(name="outbuf", bufs=2) as outbuf,
            tc.tile_pool(name="xT", bufs=2) as xTp,
            tc.tile_pool(name="qtc", bufs=2) as qtcp,
            tc.tile_pool(name="ctxc", bufs=2) as ctxcp,
            tc.tile_pool(name="proj_ps", bufs=2, space="PSUM") as proj_ps,
            tc.tile_pool(name="sc_ps", bufs=2, space="PSUM") as sc_ps,
            tc.tile_pool(name="pv_ps", bufs=2, space="PSUM") as pv_ps,
            tc.tile_pool(name="worka", bufs=2) as worka,
            tc.tile_pool(name="workb", bufs=4) as workb,
            tc.tile_pool(name="norm", bufs=2) as normp,
            tc.tile_pool(name="sums", bufs=1) as sumsp,
        ):
            # ---- loop-invariant consts ----
            wk_sb = consts.tile([128, ES, DL], bf16, tag="wk")
            nc.sync.dma_start(
                out=wk_sb[:],
                in_=wk_d.ap().rearrange("(es p) d -> p es d", p=128))
            bq_sb = consts.tile([128, NP], f32, tag="bq")
            nc.sync.dma_start(
                out=bq_sb[:],
                in_=bq_d.ap().rearrange("(np p) -> p np", p=128))
            wv_sb = consts.tile([128, ES, DL], bf16, tag="wv")
            nc.scalar.dma_start(
                out=wv_sb[:],
                in_=wv_d.ap().rearrange("(es p) d -> p es d", p=128))
            wq_sb = consts.tile([128, ES, DL], bf16, tag="wq")
            nc.scalar.dma_start(
                out=wq_sb[:],
                in_=wq_d.ap().rearrange("(es p) d -> p es d", p=128))
            wo_sb = consts.tile([128, NP, E], bf16, tag="wo")
            nc.scalar.dma_start(
                out=wo_sb[:],
                in_=wo_d.ap().rearrange("(np p) e -> p np e", p=128))
            expbiasT = consts.tile([128, ST, S], bf16, tag="expbiasT")
            bts = []
            for tt in range(ST):
                bt = xTp.tile([128, S], bf16, tag="xt", name=f"bt_{tt}")
                nc.scalar.dma_start(
                    out=bt[:],
                    in_=biasT_d.ap()[tt * 128:(tt + 1) * 128, :])
                bts.append(bt)
                if tt >= 1:
                    nc.scalar.activation(
                        out=expbiasT[:, tt - 1, :], in_=bts[tt - 1][:],
                        func=Exp)
            nc.scalar.activation(
                out=expbiasT[:, ST - 1, :], in_=bts[ST - 1][:], func=Exp)

            v_sb = persist.tile([128, ST, HL * 65], bf16, tag="v_sb")
            nc.vector.memset(
                v_sb[:].rearrange("p t (h c) -> p t h c", h=HL)
                [:, :, :, 64:65], 1.0)

            kT2_tiles = []
            for _i in range(min(repeat, 2)):
                kT2_buf = kT2p.tile([128, NP, S], bf16, tag="kT2",
                                    name=f"kT2_{_i}")
                kT2_tiles.append(kT2_buf)

            # ---- per-pass building blocks ----
            def load_strip(eng, src, qt):
                xt = xTp.tile([128, ES, 512], bf16, tag="xt")
                eng.dma_start(
                    out=xt[:],
                    in_=src.ap().rearrange("(es p) s -> p es s", p=128)
                    [:, :, qt * 512:(qt + 1) * 512])
                return xt

            def make_kproj_fillers(kT2_next):
                """20 fillers: per chunk, a strip DMA + 4 pair projections."""
                state = {}

                def load_chunk(qt):
                    def f():
                        state[qt] = load_strip(nc.sync, kT_d, qt)
                    return f

                def proj(qt, p):
                    def f():
                        xt = state[qt]
                        ps = proj_ps.tile([128, 512], f32, tag="pps")
                        for es in range(ES):
                            nc.tensor.matmul(
                                ps[:],
                                lhsT=wk_sb[:, es, p * 128:(p + 1) * 128],
                                rhs=xt[:, es, :],
                                start=(es == 0), stop=(es == ES - 1))
                        nc.vector.tensor_copy(
                            out=kT2_next[:, p, qt * 512:(qt + 1) * 512],
                            in_=ps[:])
                    return f

                fillers = []
                for qt in range(SC):
                    fillers.append(load_chunk(qt))
                    for p in range(NP):
                        fillers.append(proj(qt, p))
                return fillers

            def make_vproj_fillers():
                """20 labeled fillers: per chunk, a strip DMA + 4 v-tile
                projections. Labels let the caller compute PV deadlines."""
                state = {}

                def load_chunk(c):
                    def f():
                        state[c] = load_strip(nc.gpsimd, vT_d, c)
                    return f

                def proj(gt):
                    def f():
                        xt = state[gt // 4]
                        tl = gt % 4
                        ps = proj_ps.tile([128, 512], f32, tag="pps")
                        for es in range(ES):
                            nc.tensor.matmul(
                                ps[:],
                                lhsT=xt[:, es, tl * 128:(tl + 1) * 128],
                                rhs=wv_sb[:, es, :],
                                start=(es == 0), stop=(es == ES - 1))
                        nc.vector.tensor_copy(
                            out=v_sb[:, gt, :].rearrange(
                                "p (h c) -> p h c", h=HL)[:, :, 0:64],
                            in_=ps[:].rearrange("p (h d) -> p h d", h=HL))
                    return f

                fillers = []
                for c in range(SC):
                    fillers.append((("vload", c), load_chunk(c)))
                    for tl in range(4):
                        gt = c * 4 + tl
                        fillers.append((("vproj", gt), proj(gt)))
                return fillers

            def make_qproj_fillers(sc):
                xt = load_strip(nc.gpsimd, qT_d, sc)
                qtc = qtcp.tile([128, NP, 512], bf16, tag="qtc")

                def mk(p):
                    def f():
                        ps = proj_ps.tile([128, 512], f32, tag="pps")
                        for es in range(ES):
                            nc.tensor.matmul(
                                ps[:],
                                lhsT=wq_sb[:, es, p * 128:(p + 1) * 128],
                                rhs=xt[:, es, :],
                                start=(es == 0), stop=(es == ES - 1))
                        nc.vector.tensor_scalar_add(
                            out=qtc[:, p, :], in0=ps[:],
                            scalar1=bq_sb[:, p:p + 1])
                    return f

                return qtc, [mk(p) for p in range(NP)]

            def pops_for(n_fillers, deadlines):
                """pops[tt] schedule: deadlines = [(idx, tt)] meaning
                fillers[0..idx] must have popped by the START of iteration
                tt. Spread the rest uniformly; total == n_fillers."""
                need = [0] * ST
                for idx, tt in deadlines:
                    tt = min(max(tt, 0), ST - 1)
                    need[tt] = max(need[tt], idx + 1)
                pops = []
                cum = 0
                run = 0
                for tt in range(ST):
                    run = max(run, need[tt])
                    target = max(run, (n_fillers * (tt + 1) + ST - 1) // ST)
                    target = min(target, n_fillers)
                    pops.append(target - cum)
                    cum = target
                return pops

            def one_pass(pass_i, carry):
                kT2 = kT2_tiles[pass_i % len(kT2_tiles)]
                kT2_next = (kT2_tiles[(pass_i + 1) % len(kT2_tiles)]
                            if pass_i + 1 < repeat else None)

                if pass_i == 0:
                    # prologue: serial k projection for the first pass
                    for f in make_kproj_fillers(kT2):
                        f()

                ctx_tiles = {}

                def make_outproj_fillers(sc):
                    ctxc = ctx_tiles.pop(sc)

                    def mk(m, eh):
                        def f():
                            sm = sc * 4 + m
                            po = proj_ps.tile([128, 512], f32, tag="pps")
                            for p in range(NP):
                                nc.tensor.matmul(
                                    po[:],
                                    lhsT=ctxc[:, p, m * 128:(m + 1) * 128],
                                    rhs=wo_sb[:, p,
                                              eh * 512:(eh + 1) * 512],
                                    start=(p == 0), stop=(p == NP - 1))
                            ob = outbuf.tile([128, 512], f32, tag="ob")
                            nc.vector.tensor_copy(out=ob[:], in_=po[:])
                            nc.sync.dma_start(
                                out=out_d.ap()[sm * 128:(sm + 1) * 128,
                                               eh * 512:(eh + 1) * 512],
                                in_=ob[:])
                        return f

                    return [mk(m, eh) for m in range(4) for eh in range(2)]

                def pair(sc, p, qtc, ctxc, fillers, pops):
                    """pops[tt] = fillers to pop at the START of iteration
                    tt; all remaining fillers are drained after the loop,
                    before the pending PV drain (so filler-produced operands
                    are always emitted before their consumers)."""
                    pv0 = pv_ps.tile([65, 512], f32, tag="pv")
                    pv1 = pv_ps.tile([65, 512], f32, tag="pv")
                    pending = deque()

                    def emit_pv(ptt, pexp):
                        for hh, pv in ((0, pv0), (1, pv1)):
                            h = 2 * p + hh
                            nc.tensor.matmul(
                                pv[:],
                                lhsT=v_sb[:, ptt, h * 65:(h + 1) * 65],
                                rhs=pexp[:, hh * 512:(hh + 1) * 512],
                                start=(ptt == 0), stop=(ptt == ST - 1))

                    for tt in range(ST):
                        for _ in range(pops[tt]):
                            if fillers:
                                fillers.popleft()()
                        scp = sc_ps.tile([128, 1024], f32, tag="scp")
                        for hh in range(2):
                            nc.tensor.matmul(
                                scp[:, hh * 512:(hh + 1) * 512],
                                lhsT=kT2[hh * 64:(hh + 1) * 64, p,
                                         tt * 128:(tt + 1) * 128],
                                rhs=qtc[hh * 64:(hh + 1) * 64, p, :],
                                start=True, stop=True)
                        expt = worka.tile([128, 1024], bf16, tag="expt")
                        nc.scalar.activation(
                            out=expt[:], in_=scp[:], func=Exp, scale=0.125)
                        exptb = workb.tile([128, 1024], bf16, tag="exptb")
                        eb = expbiasT[:, tt, sc * 512:(sc + 1) * 512]
                        # same bias slice for both heads of the pair:
                        # step-0 repeat AP covers the packed pair in one op
                        eb_rep = bass.AP(
                            tensor=eb.tensor, offset=eb.offset,
                            ap=[list(eb.ap[0]), [0, 2], [1, 512]])
                        nc.vector.tensor_mul(
                            out=exptb[:], in0=expt[:], in1=eb_rep)
                        pending.append((tt, exptb))
                        if len(pending) > 2:
                            emit_pv(*pending.popleft())

                    while pending:
                        emit_pv(*pending.popleft())

                    # normalization: sums to SBUF, one fast reciprocal,
                    # per-head partition-broadcast, then multiply the
                    # unnormalized ctx rows straight out of PSUM into ctxc
                    sums_p = sumsp.tile([1, 1024], f32, tag="sums")
                    nc.vector.tensor_copy(
                        out=sums_p[0:1, 0:512], in_=pv0[64:65, :])
                    nc.vector.tensor_copy(
                        out=sums_p[0:1, 512:1024], in_=pv1[64:65, :])
                    recip_p = sumsp.tile([1, 1024], f32, tag="recip")
                    nc.vector.reciprocal_approx_fast(
                        out=recip_p[:], in_=sums_p[:])
                    rb0 = normp.tile([64, 512], f32, tag="rb")
                    nc.gpsimd.partition_broadcast(
                        out_ap=rb0[:], in_ap=recip_p[0:1, 0:512])
                    rb1 = normp.tile([64, 512], f32, tag="rb")
                    nc.gpsimd.partition_broadcast(
                        out_ap=rb1[:], in_ap=recip_p[0:1, 512:1024])
                    nc.vector.tensor_mul(
                        out=ctxc[0:64, p, :], in0=pv0[0:64, :], in1=rb0[:])
                    nc.vector.tensor_mul(
                        out=ctxc[64:128, p, :], in0=pv1[0:64, :], in1=rb1[:])

                PACE4 = [0, 0, 0, 1, 0, 0, 0, 1, 0, 0, 0, 1, 0, 0, 0, 0]

                qtc = carry.pop("qtc0", None)
                if qtc is None:
                    qtc, q0f = make_qproj_fillers(0)
                    for f in q0f:
                        f()
                for sc in range(SC):
                    ctxc = ctxcp.tile([128, NP, 512], bf16, tag="ctxc")
                    ctx_tiles[sc] = ctxc
                    fillers = deque()
                    next_qtc = None
                    if sc == 0:
                        # pair(0,0): v projection + previous pass's final
                        # outproj; pops keep v tile j emitted before PV(j)
                        labeled = make_vproj_fillers()
                        deadlines = [(i, j + 2)
                                     for i, (lab, _) in enumerate(labeled)
                                     if lab[0] == "vproj"
                                     for j in [lab[1]]]
                        mixed = deque(f for _, f in labeled)
                        mixed.extend(carry.pop("outproj3", []))
                        pops0 = pops_for(len(mixed), deadlines)
                        for p in range(NP):
                            if p == 0:
                                pair(sc, p, qtc, ctxc, mixed, pops0)
                                assert not mixed, "sc0 fillers must drain"
                            else:
                                if p == 1:
                                    next_qtc, qf = make_qproj_fillers(1)
                                    fillers.extend(qf)
                                pair(sc, p, qtc, ctxc, fillers, PACE4)
                    else:
                        if sc < SC - 1:
                            next_qtc, qf = make_qproj_fillers(sc + 1)
                            fillers.extend(qf)
                        fillers.extend(make_outproj_fillers(sc - 1))
                        if sc == SC - 1 and kT2_next is not None:
                            kf = make_kproj_fillers(kT2_next)
                            # interleave: k fillers are PE-heavy, spread them
                            mix = deque()
                            of = list(fillers)
                            ki = 0
                            for x in of:
                                mix.append(x)
                                if ki < len(kf):
                                    mix.append(kf[ki])
                                    ki += 1
                            while ki < len(kf):
                                mix.append(kf[ki])
                                ki += 1
                            # next pass's q0 projection rides the tail too
                            nqtc, q0f = make_qproj_fillers(0)
                            carry["qtc0"] = nqtc
                            mix.extend(q0f)
                            fillers = mix
                            pops = [0, 1] * 8
                        else:
                            pops = PACE4
                        for p in range(NP):
                            pair(sc, p, qtc, ctxc, fillers, pops)
                    while fillers:
                        fillers.popleft()()
                    qtc = next_qtc
                if kT2_next is None:
                    for f in make_outproj_fillers(SC - 1):
                        f()
                else:
                    carry["outproj3"] = make_outproj_fillers(SC - 1)

            carry = {}
            for _rep in range(repeat):
                one_pass(_rep, carry)

    nc.compile()
    return nc


def shard_inputs(inputs):
    """Full inputs -> per-core in_maps. Host does layout prep only: bf16
    casts, transposes of x (to [E,S]) and bias (to bias^T), and the per-core
    head-group slicing of the stacked weights. bk/bv are dropped (bk cancels
    in softmax; bv folds into the host-side bo add)."""
    import ml_dtypes
    bf = ml_dtypes.bfloat16
    ins = {k: np.asarray(v, dtype=np.float32) for k, v in inputs.items()}
    biasT = np.ascontiguousarray(ins["attention_bias"].T).astype(bf)
    xT = {}
    for name in ("query", "key", "value"):
        xT[name] = [np.ascontiguousarray(ins[name][b].T).astype(bf)
                    for b in range(B)]
    wg = {}
    for g in range(2):
        hs = slice(g * HL, (g + 1) * HL)
        wg[g] = {
            "wq": np.ascontiguousarray(
                ins["Wq"][hs].transpose(1, 0, 2).reshape(E, DL)).astype(bf),
            "wk": np.ascontiguousarray(
                ins["Wk"][hs].transpose(1, 0, 2).reshape(E, DL)).astype(bf),
            "wv": np.ascontiguousarray(
                ins["Wv"][hs].transpose(1, 0, 2).reshape(E, DL)).astype(bf),
            "wo": np.ascontiguousarray(
                ins["Wo"][g * DL:(g + 1) * DL]).astype(bf),
            "bq": np.ascontiguousarray(ins["bq"][hs].reshape(DL)),
        }
    in_maps = []
    for c in range(N_CORES):
        b, g = c // 2, c % 2
        m = {
            "qt": xT["query"][b],
            "kt": xT["key"][b],
            "vt": xT["value"][b],
            "biast": biasT,
        }
        m.update(wg[g])
        in_maps.append(m)
    return in_maps


def kernel(**inputs):
    from concourse.bass_utils import run_bass_kernel_spmd

    nc = _NC_CACHE.get("nc")
    if nc is None:
        nc = _NC_CACHE["nc"] = build_nc()

    in_maps = shard_inputs(inputs)
    res = run_bass_kernel_spmd(nc, in_maps, core_ids=list(range(N_CORES)))
    parts = [r["out"] for r in res.results]

    bo = np.asarray(inputs["bo"], dtype=np.float32)
    bv = np.asarray(inputs["bv"], dtype=np.float32)
    Wo = np.asarray(inputs["Wo"], dtype=np.float32)
    bo_eff = bo + bv.reshape(-1) @ Wo
    out = np.empty((B, S, E), np.float32)
    for b in range(B):
        out[b] = parts[2 * b] + parts[2 * b + 1] + bo_eff[None, :]
    return out


# revision 17
# speedup vs baseline: 1.2385x; 1.2385x over previous
"""Trainium2 Bass kernel for nn_MultiHeadAttention_82446192214635 (v3).

Full inputs in, full output out. Sharding: 8 cores = 4 batches x 2 head-groups
(8 heads each). Each core computes its batch's attention for its 8 heads plus
the partial output projection; host sums the two head-group partials per batch
and adds bo (plus the folded bv@Wo term).

v4 changes vs the 682us v2 baseline:
  - Cross-pass pipelining: all tile pools are opened once (outside the
    repeat loop); kT2 is double-buffered, and pass n+1's k projection,
    q0 projection, and pass n's final outproj are emitted as PE fillers
    across the pass seam (k/q0 inside pass n's last s-chunk, outproj
    inside pass n+1's first pair). The serial per-pass head/tail with
    ACT idle disappears in steady state.
  - v projection is emitted as deadline-paced fillers inside the first
    attention pair of each pass (PV lags QK by 2 tiles; pops_for keeps
    the v tile for PV(tt) emitted before it is consumed).
  - Flat PV pipeline: the 2-tile exp->PV lag spans pair and s-chunk
    boundaries (pv_ps bufs=3, proj_ps bufs=1), so the ACT exp stream
    never waits for a pair's PV tail + normalization drain.
  - bk is dropped: scores' q~.bk term is constant over keys and cancels
    in softmax (exact). bv is dropped on device: attention weights sum
    to 1, so bv contributes bv@Wo to the output, folded into bo on the
    host (exact). k/v projections drain PSUM with a plain tensor_copy.
  - Normalization drain: ctx rows copied out first so the pv banks free
    early; softmax sums rows copied on ACT (Copy is in every activation
    table set; never switch away from the exp set).

Kept from v2: host-side bf16 casts and transposes, loop-invariant consts
loaded once, exp(bias^T) precomputed once, step-0 repeat AP for the pair
bias multiply, DMA queue split (sync/scalar/gpsimd),
reciprocal_approx_fast + gpsimd partition_broadcast normalization
(custom DVE recip reads garbage from PSUM on HW; gpsimd cannot read
PSUM).
"""

import numpy as np

B, S, E = 4, 2048, 1024
H, DH = 16, 64
HL = 8          # heads per core
DL = HL * DH    # 512
N_CORES = 8
ST = S // 128   # 16 t-tiles
ES = E // 128   # 8 e-strips
SC = S // 512   # 4 s-chunks
NP = HL // 2    # 4 head pairs

_NC_CACHE = {}


def build_nc(repeat=1):
    from collections import deque
    import concourse.bass as bass
    import concourse.tile as tile
    from concourse import bacc, mybir

    f32 = mybir.dt.float32
    bf16 = mybir.dt.bfloat16
    Exp = mybir.ActivationFunctionType.Exp

    nc = bacc.Bacc("TRN2", target_bir_lowering=False, debug=False,
                   num_devices=N_CORES)

    qT_d = nc.dram_tensor("qt", [E, S], bf16, kind="ExternalInput")
    kT_d = nc.dram_tensor("kt", [E, S], bf16, kind="ExternalInput")
    vT_d = nc.dram_tensor("vt", [E, S], bf16, kind="ExternalInput")
    biasT_d = nc.dram_tensor("biast", [S, S], bf16, kind="ExternalInput")
    wq_d = nc.dram_tensor("wq", [E, DL], bf16, kind="ExternalInput")
    wk_d = nc.dram_tensor("wk", [E, DL], bf16, kind="ExternalInput")
    wv_d = nc.dram_tensor("wv", [E, DL], bf16, kind="ExternalInput")
    wo_d = nc.dram_tensor("wo", [DL, E], bf16, kind="ExternalInput")
    bq_d = nc.dram_tensor("bq", [DL], f32, kind="ExternalInput")
    out_d = nc.dram_tensor("out", [S, E], f32, kind="ExternalOutput")

    with tile.TileContext(nc) as tc:
        with (
            tc.tile_pool(name="consts", bufs=1) as consts,
            tc.tile_pool(name="persist", bufs=1) as persist,
            tc.tile_pool(name="kT2p", bufs=2) as kT2p,
            tc.tile_pool(name="outbuf", bufs=2) as outbuf,
            tc.tile_pool(name="xT", bufs=2) as xTp,
            tc.tile_pool(name="qtc", bufs=2) as qtcp,
            tc.tile_pool(name="ctxc", bufs=2) as ctxcp,
            tc.tile_pool(name="proj_ps", bufs=1, space="PSUM") as proj_ps,
            tc.tile_pool(name="sc_ps", bufs=2, space="PSUM") as sc_ps,
            tc.tile_pool(name="pv_ps", bufs=3, space="PSUM") as pv_ps,
            tc.tile_pool(name="worka", bufs=2) as worka,
            tc.tile_pool(name="workb", bufs=4) as workb,
            tc.tile_pool(name="norm", bufs=2) as normp,
            tc.tile_pool(name="sums", bufs=1) as sumsp,
        ):
            # ---- loop-invariant consts ----
            wk_sb = consts.tile([128, ES, DL], bf16, tag="wk")
            nc.sync.dma_start(
                out=wk_sb[:],
                in_=wk_d.ap().rearrange("(es p) d -> p es d", p=128))
            bq_sb = consts.tile([128, NP], f32, tag="bq")
            nc.sync.dma_start(
                out=bq_sb[:],
                in_=bq_d.ap().rearrange("(np p) -> p np", p=128))
            wv_sb = consts.tile([128, ES, DL], bf16, tag="wv")
            nc.scalar.dma_start(
                out=wv_sb[:],
                in_=wv_d.ap().rearrange("(es p) d -> p es d", p=128))
            wq_sb = consts.tile([128, ES, DL], bf16, tag="wq")
            nc.scalar.dma_start(
                out=wq_sb[:],
                in_=wq_d.ap().rearrange("(es p) d -> p es d", p=128))
            wo_sb = consts.tile([128, NP, E], bf16, tag="wo")
            nc.scalar.dma_start(
                out=wo_sb[:],
                in_=wo_d.ap().rearrange("(np p) e -> p np e", p=128))
            expbiasT = consts.tile([128, ST, S], bf16, tag="expbiasT")
            bts = []
            for tt in range(ST):
                bt = xTp.tile([128, S], bf16, tag="xt", name=f"bt_{tt}")
                nc.scalar.dma_start(
                    out=bt[:],
                    in_=biasT_d.ap()[tt * 128:(tt + 1) * 128, :])
                bts.append(bt)
                if tt >= 1:
                    nc.scalar.activation(
                        out=expbiasT[:, tt - 1, :], in_=bts[tt - 1][:],
                        func=Exp)
            nc.scalar.activation(
                out=expbiasT[:, ST - 1, :], in_=bts[ST - 1][:], func=Exp)

            v_sb = persist.tile([128, ST, HL * 65], bf16, tag="v_sb")
            nc.vector.memset(
                v_sb[:].rearrange("p t (h c) -> p t h c", h=HL)
                [:, :, :, 64:65], 1.0)

            kT2_tiles = []
            for _i in range(min(repeat, 2)):
                kT2_buf = kT2p.tile([128, NP, S], bf16, tag="kT2",
                                    name=f"kT2_{_i}")
                kT2_tiles.append(kT2_buf)

            # ---- per-pass building blocks ----
            def load_strip(eng, src, qt):
                xt = xTp.tile([128, ES, 512], bf16, tag="xt")
                eng.dma_start(
                    out=xt[:],
                    in_=src.ap().rearrange("(es p) s -> p es s", p=128)
                    [:, :, qt * 512:(qt + 1) * 512])
                return xt

            def make_kproj_fillers(kT2_next):
                """20 fillers: per chunk, a strip DMA + 4 pair projections."""
                state = {}

                def load_chunk(qt):
                    def f():
                        state[qt] = load_strip(nc.sync, kT_d, qt)
                    return f

                def proj(qt, p):
                    def f():
                        xt = state[qt]
                        ps = proj_ps.tile([128, 512], f32, tag="pps")
                        for es in range(ES):
                            nc.tensor.matmul(
                                ps[:],
                                lhsT=wk_sb[:, es, p * 128:(p + 1) * 128],
                                rhs=xt[:, es, :],
                                start=(es == 0), stop=(es == ES - 1))
                        nc.vector.tensor_copy(
                            out=kT2_next[:, p, qt * 512:(qt + 1) * 512],
                            in_=ps[:])
                    return f

                fillers = []
                for qt in range(SC):
                    fillers.append(load_chunk(qt))
                    for p in range(NP):
                        fillers.append(proj(qt, p))
                return fillers

            def make_vproj_fillers():
                """20 labeled fillers: per chunk, a strip DMA + 4 v-tile
                projections. Labels let the caller compute PV deadlines."""
                state = {}

                def load_chunk(c):
                    def f():
                        state[c] = load_strip(nc.gpsimd, vT_d, c)
                    return f

                def proj(gt):
                    def f():
                        xt = state[gt // 4]
                        tl = gt % 4
                        ps = proj_ps.tile([128, 512], f32, tag="pps")
                        for es in range(ES):
                            nc.tensor.matmul(
                                ps[:],
                                lhsT=xt[:, es, tl * 128:(tl + 1) * 128],
                                rhs=wv_sb[:, es, :],
                                start=(es == 0), stop=(es == ES - 1))
                        nc.vector.tensor_copy(
                            out=v_sb[:, gt, :].rearrange(
                                "p (h c) -> p h c", h=HL)[:, :, 0:64],
                            in_=ps[:].rearrange("p (h d) -> p h d", h=HL))
                    return f

                fillers = []
                for c in range(SC):
                    fillers.append((("vload", c), load_chunk(c)))
                    for tl in range(4):
                        gt = c * 4 + tl
                        fillers.append((("vproj", gt), proj(gt)))
                return fillers

            def make_qproj_fillers(sc):
                xt = load_strip(nc.gpsimd, qT_d, sc)
                qtc = qtcp.tile([128, NP, 512], bf16, tag="qtc")

                def mk(p):
                    def f():
                        ps = proj_ps.tile([128, 512], f32, tag="pps")
                        for es in range(ES):
                            nc.tensor.matmul(
                                ps[:],
                                lhsT=wq_sb[:, es, p * 128:(p + 1) * 128],
                                rhs=xt[:, es, :],
                                start=(es == 0), stop=(es == ES - 1))
                        nc.vector.tensor_scalar_add(
                            out=qtc[:, p, :], in0=ps[:],
                            scalar1=bq_sb[:, p:p + 1])
                    return f

                return qtc, [mk(p) for p in range(NP)]

            def pops_for(n_fillers, deadlines):
                """pops[tt] schedule: deadlines = [(idx, tt)] meaning
                fillers[0..idx] must have popped by the START of iteration
                tt. Spread the rest uniformly; total == n_fillers."""
                need = [0] * ST
                for idx, tt in deadlines:
                    tt = min(max(tt, 0), ST - 1)
                    need[tt] = max(need[tt], idx + 1)
                pops = []
                cum = 0
                run = 0
                for tt in range(ST):
                    run = max(run, need[tt])
                    target = max(run, (n_fillers * (tt + 1) + ST - 1) // ST)
                    target = min(target, n_fillers)
                    pops.append(target - cum)
                    cum = target
                return pops

            def one_pass(pass_i, carry):
                kT2 = kT2_tiles[pass_i % len(kT2_tiles)]
                kT2_next = (kT2_tiles[(pass_i + 1) % len(kT2_tiles)]
                            if pass_i + 1 < repeat else None)

                if pass_i == 0:
                    # prologue: serial k projection for the first pass
                    for f in make_kproj_fillers(kT2):
                        f()

                ctx_tiles = {}

                def make_outproj_fillers(sc):
                    ctxc = ctx_tiles.pop(sc)

                    def mk(m, eh):
                        def f():
                            sm = sc * 4 + m
                            po = proj_ps.tile([128, 512], f32, tag="pps")
                            for p in range(NP):
                                nc.tensor.matmul(
                                    po[:],
                                    lhsT=ctxc[:, p, m * 128:(m + 1) * 128],
                                    rhs=wo_sb[:, p,
                                              eh * 512:(eh + 1) * 512],
                                    start=(p == 0), stop=(p == NP - 1))
                            ob = outbuf.tile([128, 512], f32, tag="ob")
                            nc.vector.tensor_copy(out=ob[:], in_=po[:])
                            nc.sync.dma_start(
                                out=out_d.ap()[sm * 128:(sm + 1) * 128,
                                               eh * 512:(eh + 1) * 512],
                                in_=ob[:])
                        return f

                    return [mk(m, eh) for m in range(4) for eh in range(2)]

                # flat PV pipeline across pair/sc boundaries: entries carry
                # their own pv tiles so pair p+1's QK/exp stream starts while
                # pair p's last PVs and drain are still in flight
                pending = deque()

                def drain_pair(p, pv0, pv1, ctxc):
                    # copy ctx rows out first so the pv banks free early
                    # (pv pool bufs=3: the next pair's accumulators need one)
                    ctxun0 = normp.tile([64, 512], bf16, tag="ctxun")
                    nc.vector.tensor_copy(out=ctxun0[:], in_=pv0[0:64, :])
                    ctxun1 = normp.tile([64, 512], bf16, tag="ctxun")
                    nc.vector.tensor_copy(out=ctxun1[:], in_=pv1[0:64, :])
                    # sums rows on ACT (Copy is in every table set)
                    sums_p = sumsp.tile([1, 1024], f32, tag="sums")
                    nc.scalar.copy(out=sums_p[0:1, 0:512], in_=pv0[64:65, :])
                    nc.scalar.copy(
                        out=sums_p[0:1, 512:1024], in_=pv1[64:65, :])
                    recip_p = sumsp.tile([1, 1024], f32, tag="recip")
                    nc.vector.reciprocal_approx_fast(
                        out=recip_p[:], in_=sums_p[:])
                    rb0 = normp.tile([64, 512], f32, tag="rb")
                    nc.gpsimd.partition_broadcast(
                        out_ap=rb0[:], in_ap=recip_p[0:1, 0:512])
                    rb1 = normp.tile([64, 512], f32, tag="rb")
                    nc.gpsimd.partition_broadcast(
                        out_ap=rb1[:], in_ap=recip_p[0:1, 512:1024])
                    nc.vector.tensor_mul(
                        out=ctxc[0:64, p, :], in0=ctxun0[:], in1=rb0[:])
                    nc.vector.tensor_mul(
                        out=ctxc[64:128, p, :], in0=ctxun1[:], in1=rb1[:])

                def flush_one():
                    p, ptt, pexp, pv0, pv1, ctxc = pending.popleft()
                    for hh, pv in ((0, pv0), (1, pv1)):
                        h = 2 * p + hh
                        nc.tensor.matmul(
                            pv[:],
                            lhsT=v_sb[:, ptt, h * 65:(h + 1) * 65],
                            rhs=pexp[:, hh * 512:(hh + 1) * 512],
                            start=(ptt == 0), stop=(ptt == ST - 1))
                    if ptt == ST - 1:
                        drain_pair(p, pv0, pv1, ctxc)

                def pair(sc, p, qtc, ctxc, fillers, pops):
                    """pops[tt] = fillers to pop at the START of iteration
                    tt (fillers feeding PV must be scheduled ahead of their
                    consumers via the pops deadlines)."""
                    pv0 = pv_ps.tile([65, 512], f32, tag="pv")
                    pv1 = pv_ps.tile([65, 512], f32, tag="pv")

                    for tt in range(ST):
                        for _ in range(pops[tt]):
                            if fillers:
                                fillers.popleft()()
                        scp = sc_ps.tile([128, 1024], f32, tag="scp")
                        for hh in range(2):
                            nc.tensor.matmul(
                                scp[:, hh * 512:(hh + 1) * 512],
                                lhsT=kT2[hh * 64:(hh + 1) * 64, p,
                                         tt * 128:(tt + 1) * 128],
                                rhs=qtc[hh * 64:(hh + 1) * 64, p, :],
                                start=True, stop=True)
                        expt = worka.tile([128, 1024], bf16, tag="expt")
                        nc.scalar.activation(
                            out=expt[:], in_=scp[:], func=Exp, scale=0.125)
                        exptb = workb.tile([128, 1024], bf16, tag="exptb")
                        eb = expbiasT[:, tt, sc * 512:(sc + 1) * 512]
                        # same bias slice for both heads of the pair:
                        # step-0 repeat AP covers the packed pair in one op
                        eb_rep = bass.AP(
                            tensor=eb.tensor, offset=eb.offset,
                            ap=[list(eb.ap[0]), [0, 2], [1, 512]])
                        nc.vector.tensor_mul(
                            out=exptb[:], in0=expt[:], in1=eb_rep)
                        pending.append((p, tt, exptb, pv0, pv1, ctxc))
                        if len(pending) > 2:
                            flush_one()

                PACE4 = [0, 0, 0, 1, 0, 0, 0, 1, 0, 0, 0, 1, 0, 0, 0, 0]

                qtc = carry.pop("qtc0", None)
                if qtc is None:
                    qtc, q0f = make_qproj_fillers(0)
                    for f in q0f:
                        f()
                for sc in range(SC):
                    ctxc = ctxcp.tile([128, NP, 512], bf16, tag="ctxc")
                    ctx_tiles[sc] = ctxc
                    fillers = deque()
                    next_qtc = None
                    if sc == 0:
                        # pair(0,0): v projection + previous pass's final
                        # outproj; pops keep v tile j emitted before PV(j)
                        labeled = make_vproj_fillers()
                        deadlines = [(i, j + 2)
                                     for i, (lab, _) in enumerate(labeled)
                                     if lab[0] == "vproj"
                                     for j in [lab[1]]]
                        mixed = deque(f for _, f in labeled)
                        mixed.extend(carry.pop("outproj3", []))
                        pops0 = pops_for(len(mixed), deadlines)
                        for p in range(NP):
                            if p == 0:
                                pair(sc, p, qtc, ctxc, mixed, pops0)
                                assert not mixed, "sc0 fillers must drain"
                            else:
                                if p == 1:
                                    next_qtc, qf = make_qproj_fillers(1)
                                    fillers.extend(qf)
                                pair(sc, p, qtc, ctxc, fillers, PACE4)
                    else:
                        if sc < SC - 1:
                            next_qtc, qf = make_qproj_fillers(sc + 1)
                            fillers.extend(qf)
                        fillers.extend(make_outproj_fillers(sc - 1))
                        if sc == SC - 1 and kT2_next is not None:
                            # next pass's q0 projection first (it holds an
                            # xT strip slot the k loads rotate through),
                            # then outproj interleaved with the k fillers
                            nqtc, q0f = make_qproj_fillers(0)
                            carry["qtc0"] = nqtc
                            kf = make_kproj_fillers(kT2_next)
                            of = list(fillers)
                            mix = deque()
                            for i in range(4):
                                mix.append(of[i])
                                mix.append(q0f[i])
                            rest = of[4:] + kf
                            for i, x in enumerate(rest):
                                mix.append(x)
                            fillers = mix
                            pops = [0, 1] * 8
                        else:
                            pops = PACE4
                        for p in range(NP):
                            pair(sc, p, qtc, ctxc, fillers, pops)
                    while fillers:
                        fillers.popleft()()
                    qtc = next_qtc
                while pending:
                    flush_one()
                if kT2_next is None:
                    for f in make_outproj_fillers(SC - 1):
                        f()
                else:
                    carry["outproj3"] = make_outproj_fillers(SC - 1)

            carry = {}
            for _rep in range(repeat):
                one_pass(_rep, carry)

    nc.compile()
    return nc


def shard_inputs(inputs):
    """Full inputs -> per-core in_maps. Host does layout prep only: bf16
    casts, transposes of x (to [E,S]) and bias (to bias^T), and the per-core
    head-group slicing of the stacked weights. bk/bv are dropped (bk cancels
    in softmax; bv folds into the host-side bo add)."""
    import ml_dtypes
    bf = ml_dtypes.bfloat16
    ins = {k: np.asarray(v, dtype=np.float32) for k, v in inputs.items()}
    biasT = np.ascontiguousarray(ins["attention_bias"].T).astype(bf)
    xT = {}
    for name in ("query", "key", "value"):
        xT[name] = [np.ascontiguousarray(ins[name][b].T).astype(bf)
                    for b in range(B)]
    wg = {}
    for g in range(2):
        hs = slice(g * HL, (g + 1) * HL)
        wg[g] = {
            "wq": np.ascontiguousarray(
                ins["Wq"][hs].transpose(1, 0, 2).reshape(E, DL)).astype(bf),
            "wk": np.ascontiguousarray(
                ins["Wk"][hs].transpose(1, 0, 2).reshape(E, DL)).astype(bf),
            "wv": np.ascontiguousarray(
                ins["Wv"][hs].transpose(1, 0, 2).reshape(E, DL)).astype(bf),
            "wo": np.ascontiguousarray(
                ins["Wo"][g * DL:(g + 1) * DL]).astype(bf),
            "bq": np.ascontiguousarray(ins["bq"][hs].reshape(DL)),
        }
    in_maps = []
    for c in range(N_CORES):
        b, g = c // 2, c % 2
        m = {
            "qt": xT["query"][b],
            "kt": xT["key"][b],
            "vt": xT["value"][b],
            "biast": biasT,
        }
        m.update(wg[g])
        in_maps.append(m)
    return in_maps


def kernel(**inputs):
    from concourse.bass_utils import run_bass_kernel_spmd

    nc = _NC_CACHE.get("nc")
    if nc is None:
        nc = _NC_CACHE["nc"] = build_nc()

    in_maps = shard_inputs(inputs)
    res = run_bass_kernel_spmd(nc, in_maps, core_ids=list(range(N_CORES)))
    parts = [r["out"] for r in res.results]

    bo = np.asarray(inputs["bo"], dtype=np.float32)
    bv = np.asarray(inputs["bv"], dtype=np.float32)
    Wo = np.asarray(inputs["Wo"], dtype=np.float32)
    bo_eff = bo + bv.reshape(-1) @ Wo
    out = np.empty((B, S, E), np.float32)
    for b in range(B):
        out[b] = parts[2 * b] + parts[2 * b + 1] + bo_eff[None, :]
    return out


# revision 19
# speedup vs baseline: 1.2394x; 1.0007x over previous
"""Trainium2 Bass kernel for nn_MultiHeadAttention_82446192214635 (v5).

Full inputs in, full output out. Sharding: 8 cores = 4 batches x 2 head-groups
(8 heads each). Each core computes its batch's attention for its 8 heads plus
the partial output projection; host sums the two head-group partials per batch
and adds bo (plus the folded bv@Wo term).

Changes vs the 682us v2 baseline:
  - Cross-pass pipelining: all tile pools are opened once (outside the
    repeat loop); kT2 is double-buffered, and pass n+1's k projection,
    q0 projection, and pass n's final outproj are emitted as PE fillers
    across the pass seam (k/q0 inside pass n's last s-chunk, outproj
    inside pass n+1's first pair). The serial per-pass head/tail with
    ACT idle disappears in steady state.
  - v projection is emitted as deadline-paced fillers inside the first
    attention pair of each pass (PV lags QK by 2 tiles; pops_for keeps
    the v tile for PV(tt) emitted before it is consumed).
  - Flat PV pipeline: the 3-tile exp->PV lag spans pair and s-chunk
    boundaries (pv_ps bufs=3, proj_ps bufs=1), so the ACT exp stream
    never waits for a pair's PV tail + normalization drain, and HW
    semaphore latencies are absorbed by the extra tile of slack.
  - bk is dropped: scores' q~.bk term is constant over keys and cancels
    in softmax (exact). bv is dropped on device: attention weights sum
    to 1, so bv contributes bv@Wo to the output, folded into bo on the
    host (exact). k/v projections drain PSUM with a plain tensor_copy.
  - Normalization drain: ctx rows copied out first so the pv banks free
    early; all drain copies stay on DVE (HW probe showed DVE has slack;
    putting them on ACT head-of-line blocks the exp stream in its
    strict-FIFO queue).

Kept from v2: host-side bf16 casts and transposes, loop-invariant consts
loaded once, exp(bias^T) precomputed once, step-0 repeat AP for the pair
bias multiply, DMA queue split (sync/scalar/gpsimd),
reciprocal_approx_fast + gpsimd partition_broadcast normalization
(custom DVE recip reads garbage from PSUM on HW; gpsimd cannot read
PSUM).
"""

import numpy as np

B, S, E = 4, 2048, 1024
H, DH = 16, 64
HL = 8          # heads per core
DL = HL * DH    # 512
N_CORES = 8
ST = S // 128   # 16 t-tiles
ES = E // 128   # 8 e-strips
SC = S // 512   # 4 s-chunks
NP = HL // 2    # 4 head pairs

_NC_CACHE = {}


def build_nc(repeat=1):
    from collections import deque
    import concourse.bass as bass
    import concourse.tile as tile
    from concourse import bacc, mybir

    f32 = mybir.dt.float32
    bf16 = mybir.dt.bfloat16
    Exp = mybir.ActivationFunctionType.Exp

    nc = bacc.Bacc("TRN2", target_bir_lowering=False, debug=False,
                   num_devices=N_CORES)

    qT_d = nc.dram_tensor("qt", [E, S], bf16, kind="ExternalInput")
    kT_d = nc.dram_tensor("kt", [E, S], bf16, kind="ExternalInput")
    vT_d = nc.dram_tensor("vt", [E, S], bf16, kind="ExternalInput")
    biasT_d = nc.dram_tensor("biast", [S, S], bf16, kind="ExternalInput")
    wq_d = nc.dram_tensor("wq", [E, DL], bf16, kind="ExternalInput")
    wk_d = nc.dram_tensor("wk", [E, DL], bf16, kind="ExternalInput")
    wv_d = nc.dram_tensor("wv", [E, DL], bf16, kind="ExternalInput")
    wo_d = nc.dram_tensor("wo", [DL, E], bf16, kind="ExternalInput")
    bq_d = nc.dram_tensor("bq", [DL], f32, kind="ExternalInput")
    out_d = nc.dram_tensor("out", [S, E], f32, kind="ExternalOutput")

    with tile.TileContext(nc) as tc:
        with (
            tc.tile_pool(name="consts", bufs=1) as consts,
            tc.tile_pool(name="persist", bufs=1) as persist,
            tc.tile_pool(name="kT2p", bufs=2) as kT2p,
            tc.tile_pool(name="outbuf", bufs=2) as outbuf,
            tc.tile_pool(name="xT", bufs=2) as xTp,
            tc.tile_pool(name="qtc", bufs=2) as qtcp,
            tc.tile_pool(name="ctxc", bufs=2) as ctxcp,
            tc.tile_pool(name="proj_ps", bufs=1, space="PSUM") as proj_ps,
            tc.tile_pool(name="sc_ps", bufs=2, space="PSUM") as sc_ps,
            tc.tile_pool(name="pv_ps", bufs=3, space="PSUM") as pv_ps,
            tc.tile_pool(name="worka", bufs=2) as worka,
            tc.tile_pool(name="workb", bufs=4) as workb,
            tc.tile_pool(name="norm", bufs=2) as normp,
            tc.tile_pool(name="sums", bufs=1) as sumsp,
        ):
            # ---- loop-invariant consts ----
            wk_sb = consts.tile([128, ES, DL], bf16, tag="wk")
            nc.sync.dma_start(
                out=wk_sb[:],
                in_=wk_d.ap().rearrange("(es p) d -> p es d", p=128))
            bq_sb = consts.tile([128, NP], f32, tag="bq")
            nc.sync.dma_start(
                out=bq_sb[:],
                in_=bq_d.ap().rearrange("(np p) -> p np", p=128))
            wv_sb = consts.tile([128, ES, DL], bf16, tag="wv")
            nc.scalar.dma_start(
                out=wv_sb[:],
                in_=wv_d.ap().rearrange("(es p) d -> p es d", p=128))
            wq_sb = consts.tile([128, ES, DL], bf16, tag="wq")
            nc.scalar.dma_start(
                out=wq_sb[:],
                in_=wq_d.ap().rearrange("(es p) d -> p es d", p=128))
            wo_sb = consts.tile([128, NP, E], bf16, tag="wo")
            nc.scalar.dma_start(
                out=wo_sb[:],
                in_=wo_d.ap().rearrange("(np p) e -> p np e", p=128))
            expbiasT = consts.tile([128, ST, S], bf16, tag="expbiasT")
            bts = []
            for tt in range(ST):
                bt = xTp.tile([128, S], bf16, tag="xt", name=f"bt_{tt}")
                nc.scalar.dma_start(
                    out=bt[:],
                    in_=biasT_d.ap()[tt * 128:(tt + 1) * 128, :])
                bts.append(bt)
                if tt >= 1:
                    nc.scalar.activation(
                        out=expbiasT[:, tt - 1, :], in_=bts[tt - 1][:],
                        func=Exp)
            nc.scalar.activation(
                out=expbiasT[:, ST - 1, :], in_=bts[ST - 1][:], func=Exp)

            v_sb = persist.tile([128, ST, HL * 65], bf16, tag="v_sb")
            nc.vector.memset(
                v_sb[:].rearrange("p t (h c) -> p t h c", h=HL)
                [:, :, :, 64:65], 1.0)

            kT2_tiles = []
            for _i in range(min(repeat, 2)):
                kT2_buf = kT2p.tile([128, NP, S], bf16, tag="kT2",
                                    name=f"kT2_{_i}")
                kT2_tiles.append(kT2_buf)

            # ---- per-pass building blocks ----
            def load_strip(eng, src, qt):
                xt = xTp.tile([128, ES, 512], bf16, tag="xt")
                eng.dma_start(
                    out=xt[:],
                    in_=src.ap().rearrange("(es p) s -> p es s", p=128)
                    [:, :, qt * 512:(qt + 1) * 512])
                return xt

            def make_kproj_fillers(kT2_next):
                """20 fillers: per chunk, a strip DMA + 4 pair projections."""
                state = {}

                def load_chunk(qt):
                    def f():
                        state[qt] = load_strip(nc.sync, kT_d, qt)
                    return f

                def proj(qt, p):
                    def f():
                        xt = state[qt]
                        ps = proj_ps.tile([128, 512], f32, tag="pps")
                        for es in range(ES):
                            nc.tensor.matmul(
                                ps[:],
                                lhsT=wk_sb[:, es, p * 128:(p + 1) * 128],
                                rhs=xt[:, es, :],
                                start=(es == 0), stop=(es == ES - 1))
                        nc.vector.tensor_copy(
                            out=kT2_next[:, p, qt * 512:(qt + 1) * 512],
                            in_=ps[:])
                    return f

                fillers = []
                for qt in range(SC):
                    fillers.append(load_chunk(qt))
                    for p in range(NP):
                        fillers.append(proj(qt, p))
                return fillers

            def make_vproj_fillers():
                """20 labeled fillers: per chunk, a strip DMA + 4 v-tile
                projections. Labels let the caller compute PV deadlines."""
                state = {}

                def load_chunk(c):
                    def f():
                        state[c] = load_strip(nc.gpsimd, vT_d, c)
                    return f

                def proj(gt):
                    def f():
                        xt = state[gt // 4]
                        tl = gt % 4
                        ps = proj_ps.tile([128, 512], f32, tag="pps")
                        for es in range(ES):
                            nc.tensor.matmul(
                                ps[:],
                                lhsT=xt[:, es, tl * 128:(tl + 1) * 128],
                                rhs=wv_sb[:, es, :],
                                start=(es == 0), stop=(es == ES - 1))
                        nc.vector.tensor_copy(
                            out=v_sb[:, gt, :].rearrange(
                                "p (h c) -> p h c", h=HL)[:, :, 0:64],
                            in_=ps[:].rearrange("p (h d) -> p h d", h=HL))
                    return f

                fillers = []
                for c in range(SC):
                    fillers.append((("vload", c), load_chunk(c)))
                    for tl in range(4):
                        gt = c * 4 + tl
                        fillers.append((("vproj", gt), proj(gt)))
                return fillers

            def make_qproj_fillers(sc):
                xt = load_strip(nc.gpsimd, qT_d, sc)
                qtc = qtcp.tile([128, NP, 512], bf16, tag="qtc")

                def mk(p):
                    def f():
                        ps = proj_ps.tile([128, 512], f32, tag="pps")
                        for es in range(ES):
                            nc.tensor.matmul(
                                ps[:],
                                lhsT=wq_sb[:, es, p * 128:(p + 1) * 128],
                                rhs=xt[:, es, :],
                                start=(es == 0), stop=(es == ES - 1))
                        nc.vector.tensor_scalar_add(
                            out=qtc[:, p, :], in0=ps[:],
                            scalar1=bq_sb[:, p:p + 1])
                    return f

                return qtc, [mk(p) for p in range(NP)]

            def pops_for(n_fillers, deadlines):
                """pops[tt] schedule: deadlines = [(idx, tt)] meaning
                fillers[0..idx] must have popped by the START of iteration
                tt. Spread the rest uniformly; total == n_fillers."""
                need = [0] * ST
                for idx, tt in deadlines:
                    tt = min(max(tt, 0), ST - 1)
                    need[tt] = max(need[tt], idx + 1)
                pops = []
                cum = 0
                run = 0
                for tt in range(ST):
                    run = max(run, need[tt])
                    target = max(run, (n_fillers * (tt + 1) + ST - 1) // ST)
                    target = min(target, n_fillers)
                    pops.append(target - cum)
                    cum = target
                return pops

            def one_pass(pass_i, carry):
                kT2 = kT2_tiles[pass_i % len(kT2_tiles)]
                kT2_next = (kT2_tiles[(pass_i + 1) % len(kT2_tiles)]
                            if pass_i + 1 < repeat else None)

                if pass_i == 0:
                    # prologue: serial k projection for the first pass
                    for f in make_kproj_fillers(kT2):
                        f()

                ctx_tiles = {}

                def make_outproj_fillers(sc):
                    ctxc = ctx_tiles.pop(sc)

                    def mk(m, eh):
                        def f():
                            sm = sc * 4 + m
                            po = proj_ps.tile([128, 512], f32, tag="pps")
                            for p in range(NP):
                                nc.tensor.matmul(
                                    po[:],
                                    lhsT=ctxc[:, p, m * 128:(m + 1) * 128],
                                    rhs=wo_sb[:, p,
                                              eh * 512:(eh + 1) * 512],
                                    start=(p == 0), stop=(p == NP - 1))
                            ob = outbuf.tile([128, 512], f32, tag="ob")
                            nc.vector.tensor_copy(out=ob[:], in_=po[:])
                            nc.sync.dma_start(
                                out=out_d.ap()[sm * 128:(sm + 1) * 128,
                                               eh * 512:(eh + 1) * 512],
                                in_=ob[:])
                        return f

                    return [mk(m, eh) for m in range(4) for eh in range(2)]

                # flat PV pipeline across pair/sc boundaries: entries carry
                # their own pv tiles so pair p+1's QK/exp stream starts while
                # pair p's last PVs and drain are still in flight
                pending = deque()

                def drain_pair(p, pv0, pv1, ctxc):
                    # copy ctx rows out first so the pv banks free early
                    # (pv pool bufs=3: the next pair's accumulators need one)
                    ctxun0 = normp.tile([64, 512], bf16, tag="ctxun")
                    nc.vector.tensor_copy(out=ctxun0[:], in_=pv0[0:64, :])
                    ctxun1 = normp.tile([64, 512], bf16, tag="ctxun")
                    nc.vector.tensor_copy(out=ctxun1[:], in_=pv1[0:64, :])
                    # sums rows on DVE (slack engine; keeps them out of
                    # the ACT FIFO where they'd head-of-line block the exps)
                    sums_p = sumsp.tile([1, 1024], f32, tag="sums")
                    nc.vector.tensor_copy(
                        out=sums_p[0:1, 0:512], in_=pv0[64:65, :])
                    nc.vector.tensor_copy(
                        out=sums_p[0:1, 512:1024], in_=pv1[64:65, :])
                    recip_p = sumsp.tile([1, 1024], f32, tag="recip")
                    nc.vector.reciprocal_approx_fast(
                        out=recip_p[:], in_=sums_p[:])
                    rb0 = normp.tile([64, 512], f32, tag="rb")
                    nc.gpsimd.partition_broadcast(
                        out_ap=rb0[:], in_ap=recip_p[0:1, 0:512])
                    rb1 = normp.tile([64, 512], f32, tag="rb")
                    nc.gpsimd.partition_broadcast(
                        out_ap=rb1[:], in_ap=recip_p[0:1, 512:1024])
                    nc.vector.tensor_mul(
                        out=ctxc[0:64, p, :], in0=ctxun0[:], in1=rb0[:])
                    nc.vector.tensor_mul(
                        out=ctxc[64:128, p, :], in0=ctxun1[:], in1=rb1[:])

                def flush_one():
                    p, ptt, pexp, pv0, pv1, ctxc = pending.popleft()
                    for hh, pv in ((0, pv0), (1, pv1)):
                        h = 2 * p + hh
                        nc.tensor.matmul(
                            pv[:],
                            lhsT=v_sb[:, ptt, h * 65:(h + 1) * 65],
                            rhs=pexp[:, hh * 512:(hh + 1) * 512],
                            start=(ptt == 0), stop=(ptt == ST - 1))
                    if ptt == ST - 1:
                        drain_pair(p, pv0, pv1, ctxc)

                def pair(sc, p, qtc, ctxc, fillers, pops):
                    """pops[tt] = fillers to pop at the START of iteration
                    tt (fillers feeding PV must be scheduled ahead of their
                    consumers via the pops deadlines)."""
                    pv0 = pv_ps.tile([65, 512], f32, tag="pv")
                    pv1 = pv_ps.tile([65, 512], f32, tag="pv")

                    for tt in range(ST):
                        for _ in range(pops[tt]):
                            if fillers:
                                fillers.popleft()()
                        scp = sc_ps.tile([128, 1024], f32, tag="scp")
                        for hh in range(2):
                            nc.tensor.matmul(
                                scp[:, hh * 512:(hh + 1) * 512],
                                lhsT=kT2[hh * 64:(hh + 1) * 64, p,
                                         tt * 128:(tt + 1) * 128],
                                rhs=qtc[hh * 64:(hh + 1) * 64, p, :],
                                start=True, stop=True)
                        expt = worka.tile([128, 1024], bf16, tag="expt")
                        nc.scalar.activation(
                            out=expt[:], in_=scp[:], func=Exp, scale=0.125)
                        exptb = workb.tile([128, 1024], bf16, tag="exptb")
                        eb = expbiasT[:, tt, sc * 512:(sc + 1) * 512]
                        # same bias slice for both heads of the pair:
                        # step-0 repeat AP covers the packed pair in one op
                        eb_rep = bass.AP(
                            tensor=eb.tensor, offset=eb.offset,
                            ap=[list(eb.ap[0]), [0, 2], [1, 512]])
                        nc.vector.tensor_mul(
                            out=exptb[:], in0=expt[:], in1=eb_rep)
                        pending.append((p, tt, exptb, pv0, pv1, ctxc))
                        if len(pending) > 3:
                            flush_one()

                PACE4 = [0, 0, 0, 1, 0, 0, 0, 1, 0, 0, 0, 1, 0, 0, 0, 0]

                qtc = carry.pop("qtc0", None)
                if qtc is None:
                    qtc, q0f = make_qproj_fillers(0)
                    for f in q0f:
                        f()
                for sc in range(SC):
                    ctxc = ctxcp.tile([128, NP, 512], bf16, tag="ctxc")
                    ctx_tiles[sc] = ctxc
                    fillers = deque()
                    next_qtc = None
                    if sc == 0:
                        # pair(0,0): v projection + previous pass's final
                        # outproj; pops keep v tile j emitted before PV(j)
                        labeled = make_vproj_fillers()
                        deadlines = [(i, j + 2)
                                     for i, (lab, _) in enumerate(labeled)
                                     if lab[0] == "vproj"
                                     for j in [lab[1]]]
                        mixed = deque(f for _, f in labeled)
                        mixed.extend(carry.pop("outproj3", []))
                        pops0 = pops_for(len(mixed), deadlines)
                        for p in range(NP):
                            if p == 0:
                                pair(sc, p, qtc, ctxc, mixed, pops0)
                                assert not mixed, "sc0 fillers must drain"
                            else:
                                if p == 1:
                                    next_qtc, qf = make_qproj_fillers(1)
                                    fillers.extend(qf)
                                pair(sc, p, qtc, ctxc, fillers, PACE4)
                    else:
                        if sc < SC - 1:
                            next_qtc, qf = make_qproj_fillers(sc + 1)
                            fillers.extend(qf)
                        fillers.extend(make_outproj_fillers(sc - 1))
                        if sc == SC - 1 and kT2_next is not None:
                            # next pass's q0 projection first (it holds an
                            # xT strip slot the k loads rotate through),
                            # then outproj interleaved with the k fillers
                            nqtc, q0f = make_qproj_fillers(0)
                            carry["qtc0"] = nqtc
                            kf = make_kproj_fillers(kT2_next)
                            of = list(fillers)
                            mix = deque()
                            for i in range(4):
                                mix.append(of[i])
                                mix.append(q0f[i])
                            rest = of[4:] + kf
                            for i, x in enumerate(rest):
                                mix.append(x)
                            fillers = mix
                            pops = [0, 1] * 8
                        else:
                            pops = PACE4
                        for p in range(NP):
                            pair(sc, p, qtc, ctxc, fillers, pops)
                    while fillers:
                        fillers.popleft()()
                    qtc = next_qtc
                while pending:
                    flush_one()
                if kT2_next is None:
                    for f in make_outproj_fillers(SC - 1):
                        f()
                else:
                    carry["outproj3"] = make_outproj_fillers(SC - 1)

            carry = {}
            for _rep in range(repeat):
                one_pass(_rep, carry)

    nc.compile()
    return nc


def shard_inputs(inputs):
    """Full inputs -> per-core in_maps. Host does layout prep only: bf16
    casts, transposes of x (to [E,S]) and bias (to bias^T), and the per-core
    head-group slicing of the stacked weights. bk/bv are dropped (bk cancels
    in softmax; bv folds into the host-side bo add)."""
    import ml_dtypes
    bf = ml_dtypes.bfloat16
    ins = {k: np.asarray(v, dtype=np.float32) for k, v in inputs.items()}
    biasT = np.ascontiguousarray(ins["attention_bias"].T).astype(bf)
    xT = {}
    for name in ("query", "key", "value"):
        xT[name] = [np.ascontiguousarray(ins[name][b].T).astype(bf)
                    for b in range(B)]
    wg = {}
    for g in range(2):
        hs = slice(g * HL, (g + 1) * HL)
        wg[g] = {
            "wq": np.ascontiguousarray(
                ins["Wq"][hs].transpose(1, 0, 2).reshape(E, DL)).astype(bf),
            "wk": np.ascontiguousarray(
                ins["Wk"][hs].transpose(1, 0, 2).reshape(E, DL)).astype(bf),
            "wv": np.ascontiguousarray(
                ins["Wv"][hs].transpose(1, 0, 2).reshape(E, DL)).astype(bf),
            "wo": np.ascontiguousarray(
                ins["Wo"][g * DL:(g + 1) * DL]).astype(bf),
            "bq": np.ascontiguousarray(ins["bq"][hs].reshape(DL)),
        }
    in_maps = []
    for c in range(N_CORES):
        b, g = c // 2, c % 2
        m = {
            "qt": xT["query"][b],
            "kt": xT["key"][b],
            "vt": xT["value"][b],
            "biast": biasT,
        }
        m.update(wg[g])
        in_maps.append(m)
    return in_maps


def kernel(**inputs):
    from concourse.bass_utils import run_bass_kernel_spmd

    nc = _NC_CACHE.get("nc")
    if nc is None:
        nc = _NC_CACHE["nc"] = build_nc()

    in_maps = shard_inputs(inputs)
    res = run_bass_kernel_spmd(nc, in_maps, core_ids=list(range(N_CORES)))
    parts = [r["out"] for r in res.results]

    bo = np.asarray(inputs["bo"], dtype=np.float32)
    bv = np.asarray(inputs["bv"], dtype=np.float32)
    Wo = np.asarray(inputs["Wo"], dtype=np.float32)
    bo_eff = bo + bv.reshape(-1) @ Wo
    out = np.empty((B, S, E), np.float32)
    for b in range(B):
        out[b] = parts[2 * b] + parts[2 * b + 1] + bo_eff[None, :]
    return out
